# revision 1
# baseline (speedup 1.0000x reference)
"""Causal self-attention TP kernel for 8 trn2 NeuronCores.

Problem shapes (hardcoded): x [2, 2048, 2048] f32, w_attn [2048, 6144],
w_proj [2048, 2048], 16 heads, head_dim 128.

Sharding: tensor-parallel over heads — core i owns heads {2i, 2i+1} for BOTH
batches. Each core computes its local-head qkv + attention, producing
y_local^T [256 feat, 4096 tok] (unnormalized) plus per-token softmax row
sums r, shipped IN-BAND with y: one AllToAll per batch re-shards 258-row
shards [128 h0-feat | 128 h1-feat | r_h0 | r_h1] x 256 tok from
feature-split to token-split. The receiver normalizes y by 1/r per head
(normalization commutes with the projection within each head block) and
projects its 512 tokens against the full w_proj.

Precision: q/k/x/w_attn stay fp32r (full PE rate); p = exp(scores), v, y, r
(transport), w_proj are bf16; row sums accumulate in f32. The causal mask is
folded into the score PSUM accumulation as an identity x (-1e9 staircase)
matmul. Row sums: P tiles accumulate on DVE/GpSimd (alternating c, separate
f32 accumulators); every 8th tile goes through a per-tile ones-matmul on the
PE; two final ones-matmuls per q-tile fold the accumulators into PSUM.
"""

import numpy as np
import ml_dtypes

import concourse.bass as bass
import concourse.mybir as mybir
import concourse.tile as tile
from concourse import bacc
from concourse.bass_utils import run_bass_kernel_spmd

F32 = mybir.dt.float32
F32R = mybir.dt.float32r
BF16 = mybir.dt.bfloat16
NPBF16 = ml_dtypes.bfloat16

B, T, C = 2, 2048, 2048
H, D = 16, 128
NTOK = B * T                     # 4096 flat tokens (batch-major)
SCALE = 1.0 / float(np.sqrt(D))  # 0.08838834764831845
NCORES = 8
HPC = H // NCORES                # 2 heads per core
FLOC = HPC * D                   # 256 local v features
QK = 512                         # q+k local features (2 heads x 128 x 2)
NEG = -1.0e9                     # causal mask bias (exp(SCALE*NEG) == 0)
SROW = 129                       # a2a shard rows: 128 feat + 1 r row

last_exec_time_ns = None
_cache = {}


def r32(ap):
    return ap.bitcast(F32R)


def _mask01_np():
    # mask[m, kk, qq] = 1 iff kk <= qq - 128*m (diagonal tile offset m)
    m = np.arange(4)[:, None, None]
    kk = np.arange(128)[None, :, None]
    qq = np.arange(512)[None, None, :]
    return (kk <= qq - 128 * m).astype(np.float32)


def build_nc(no_collective=False, reps=1):
    nc = bacc.Bacc("TRN2", target_bir_lowering=False, debug=False,
                   num_devices=1 if no_collective else NCORES)

    xt = nc.dram_tensor("xt", [C, NTOK], F32, kind="ExternalInput")
    wqk = nc.dram_tensor("wqk", [C, QK], F32, kind="ExternalInput")
    wv = nc.dram_tensor("wv", [C, FLOC], F32, kind="ExternalInput")
    wp = nc.dram_tensor("wp", [C, C], BF16, kind="ExternalInput")
    out = nc.dram_tensor("out", [512, C], F32, kind="ExternalOutput")

    # per-batch a2a buffers: 8 shards x [258 rows x 256 tok] bf16;
    # shard rows: [h0 feat 128 | r_h0 | h1 feat 128 | r_h1]
    y_loc = [nc.dram_tensor(f"y_loc{b}", [8 * 2 * SROW, 256], BF16)
             for b in range(B)]
    y_t = [nc.dram_tensor(f"y_t{b}", [8 * 2 * SROW, 256], BF16)
           for b in range(B)]
    ri_dram = [nc.dram_tensor(f"ri{b}", [1, 16, 256], BF16) for b in range(B)]
    warm = [nc.dram_tensor(f"warm{k}", [8, 2], F32) for k in range(4)]
    warm_t = [nc.dram_tensor(f"warm_t{k}", [8, 2], F32) for k in range(4)]

    maskb_dr = nc.inline_tensor(_mask01_np().astype(NPBF16), "maskb")
    ones_dr = nc.inline_tensor(np.ones((128, 1), np.float32), "ones_c")
    onesb_dr = nc.inline_tensor(np.ones((128, 1), NPBF16), "onesb_c")
    zeros_dr = nc.inline_tensor(np.zeros((128, 1), np.float32), "zeros_c")

    def a2a_op(src, dst):
        if no_collective:
            nc.gpsimd.dma_start(out=dst[:, :], in_=src[:, :])
        else:
            nc.gpsimd.collective_compute(
                "AllToAll", mybir.AluOpType.bypass,
                replica_groups=[list(range(NCORES))],
                ins=[src[:, :]], outs=[dst[:, :]],
            )

    with tile.TileContext(nc) as tc:
      for _rep in range(reps):
        # ---- persistent (qkv outputs + constants) ----
        with tc.tile_pool(name="persist", bufs=1) as persist:
            # q^T,k^T for 2 heads, all tokens: chunk f = {q_h0, q_h1, k_h0, k_h1}
            qk_res = persist.tile([128, 4, NTOK], F32R)
            ones_sb = persist.tile([128, 1], F32R)
            onesb_sb = persist.tile([128, 1], BF16)
            zeros_sb = persist.tile([128, 1], F32)
            nc.gpsimd.dma_start(out=zeros_sb, in_=zeros_dr.ap())
            maskb_sb = persist.tile([128, 4, 512], BF16)
            nc.gpsimd.dma_start(out=maskb_sb,
                                in_=maskb_dr.ap().rearrange("m p q -> p m q"))
            scr = persist.tile([128, 1], F32)
            # warm the ACT exp table set (~2.7us) before attention needs it
            nc.scalar.activation(scr, zeros_sb,
                                 mybir.ActivationFunctionType.Exp, bias=zeros_sb)
            nc.gpsimd.dma_start(out=ones_sb, in_=r32(ones_dr.ap()))
            nc.gpsimd.dma_start(out=onesb_sb, in_=onesb_dr.ap())
            # warm the collective stream with a dummy a2a (overlaps qkv);
            # more warmers are trickled through phase 1 to keep it hot
            a2a_op(warm[0], warm_t[0])

            # v for all (batch, head), bf16, SBUF-resident: [tok128, ktile, feat]
            p2v_ctx = tc.tile_pool(name="p2v", bufs=4)
            p2v = p2v_ctx.__enter__()
            v_pre = {}

            # ================= phase 1: qkv =================
            with (
                tc.tile_pool(name="p1w", bufs=1) as p1w,
                tc.tile_pool(name="p1x", bufs=3) as p1x,
                tc.tile_pool(name="p1ps", bufs=4, space="PSUM") as p1ps,
                tc.tile_pool(name="p1psv", bufs=4, space="PSUM") as p1psv,
            ):
                wqk_sb = p1w.tile([128, 16, QK], F32R)
                wv_sb = p1w.tile([128, 16, FLOC], F32R)

                for tt in range(8):          # 512-token tiles over 4096 tokens
                    b = tt // 4
                    xh = []
                    for half in range(2):
                        xbuf = p1x.tile([128, 8, 512], F32R, tag="xh")
                        c0 = half * 8
                        if tt == 0:
                            # chase: per-chunk loads, weights on sync queue,
                            # x alternating scalar/gpsimd queues (both idle)
                            for cc in range(8):
                                nc.sync.dma_start(
                                    out=wqk_sb[:, c0 + cc, :],
                                    in_=r32(wqk[(c0 + cc) * 128:(c0 + cc + 1) * 128, :]))
                                xq = nc.scalar if cc % 2 == 0 else nc.gpsimd
                                xq.dma_start(
                                    out=xbuf[:, cc, :],
                                    in_=r32(xt[(c0 + cc) * 128:(c0 + cc + 1) * 128,
                                               tt * 512:(tt + 1) * 512]))
                        else:
                            nc.sync.dma_start(
                                out=xbuf,
                                in_=r32(xt[c0 * 128:(c0 + 8) * 128,
                                           tt * 512:(tt + 1) * 512].rearrange(
                                               "(n p) f -> p n f", p=128)))
                        xh.append(xbuf)
                    if tt == 0:
                        nc.sync.dma_start(
                            out=wv_sb,
                            in_=r32(wv.ap().rearrange("(n p) f -> p n f", p=128)))
                    if tt % 4 == 0:
                        for h in range(HPC):
                            v_pre[(b, h)] = p2v.tile(
                                [128, 16, 128], BF16, tag="vsb", name=f"v{b}{h}")
                    # c-outer so compute chases the DMA stream chunk by chunk
                    ps = [p1ps.tile([128, 512], F32, tag="qkps", name=f"qk{fb}")
                          for fb in range(4)]
                    psv = [p1psv.tile([128, FLOC], F32, tag="vps", name=f"v{tb}")
                           for tb in range(4)]
                    for c in range(16):
                        half, cc = c // 8, c % 8
                        for fb in range(4):
                            nc.tensor.matmul(
                                ps[fb],
                                lhsT=wqk_sb[:, c, fb * 128:(fb + 1) * 128],
                                rhs=xh[half][:, cc, :],
                                start=(c == 0), stop=(c == 15),
                            )
                        for tb in range(4):
                            nc.tensor.matmul(
                                psv[tb],
                                lhsT=xh[half][:, cc, tb * 128:(tb + 1) * 128],
                                rhs=wv_sb[:, c, :],
                                start=(c == 0), stop=(c == 15),
                            )
                    for fb in range(4):
                        nc.vector.tensor_copy(
                            qk_res[:, fb, tt * 512:(tt + 1) * 512], ps[fb])
                    for tb in range(4):
                        for h in range(HPC):
                            nc.vector.tensor_copy(
                                v_pre[(b, h)][:, (tt % 4) * 4 + tb, :],
                                psv[tb][:, h * 128:(h + 1) * 128])

            # ============ phases 2+3: attention + per-batch a2a + proj ======
            with (
                tc.tile_pool(name="p4w", bufs=4) as p4w,
                tc.tile_pool(name="p4y", bufs=2) as p4y,
                tc.tile_pool(name="p4r", bufs=2) as p4r,
                tc.tile_pool(name="p4rb", bufs=1) as p4rb,
                tc.tile_pool(name="p4s", bufs=2) as p4s,
            ):
                wp_tiles = []

                def load_wp_chunk(ch):
                    wt = p4w.tile([128, 16, 512], BF16, tag="wp", name=f"wp{ch}")
                    nc.sync.dma_start(
                        out=wt,
                        in_=wp[:, ch * 512:(ch + 1) * 512].rearrange(
                            "(n p) f -> p n f", p=128))
                    wp_tiles.append(wt)

                yts = []

                def recv_batch(b):
                    """post-a2a: load y/r for my 512 tokens, normalize y.
                    yts chunk index = h*8 + s  (w_proj row block g=2s+h)."""
                    rr = p4r.tile([16, 256], BF16, tag="rrecv", name=f"rr{b}")
                    yb = p4y.tile([128, 16, 256], BF16, tag="yt", name=f"yt{b}")
                    src = y_t[b].ap().rearrange("(s r) t -> r s t", r=2 * SROW)
                    for h in range(HPC):
                        nc.sync.dma_start(out=rr[h * 8:(h + 1) * 8, :],
                                          in_=src[h * SROW + 128])
                        nc.sync.dma_start(out=yb[:, h * 8:(h + 1) * 8, :],
                                          in_=src[h * SROW:h * SROW + 128])
                    ri = p4r.tile([16, 256], BF16, tag="rinv", name=f"ri{b}")
                    with nc.allow_low_precision(reason="softmax denom bf16"):
                        nc.vector.reciprocal(ri, rr)
                    nc.sync.dma_start(out=ri_dram[b].ap(), in_=ri)
                    rb = p4rb.tile([128, 16, 256], BF16, tag="rb", name=f"rb{b}")
                    nc.sync.dma_start(
                        out=rb, in_=ri_dram[b].ap().to_broadcast([128, 16, 256]))
                    nc.vector.tensor_mul(yb, yb, rb)
                    yts.append(yb)

                with (
                    tc.tile_pool(name="p2p", bufs=5) as p2p,
                    tc.tile_pool(name="p2r", bufs=2) as p2r,
                    tc.tile_pool(name="p2rs", bufs=4) as p2rs,
                    tc.tile_pool(name="p2y", bufs=4) as p2y,
                    tc.tile_pool(name="p2pss", bufs=2, space="PSUM") as p2pss,
                    tc.tile_pool(name="p2psy", bufs=2, space="PSUM") as p2psy,
                    tc.tile_pool(name="p2psr", bufs=2, space="PSUM") as p2psr,
                ):
                    nwp = [0]

                    def attn_batch(b):
                        """Both heads' streams interleaved at pair granularity:
                        while one head's exp runs, the other head's matmuls
                        keep the PE busy (two independent dependency chains)."""
                        tok0 = b * T
                        steps = [(j, t) for j in range(4)
                                 for t in range((4 * j + 4) // 2)]
                        st = [dict(idx=0, pend=None, j=-1) for _ in range(HPC)]

                        def scores(h, j, t):
                            kf = 2 + h
                            qs = qk_res[:, h,
                                        tok0 + j * 512: tok0 + (j + 1) * 512]
                            s_ps = p2pss.tile([128, 2, 512], F32, tag="sps")
                            diag0 = 2 * t - 4 * j
                            for e in range(2):
                                c = 2 * t + e
                                nc.tensor.matmul(
                                    s_ps[:, e, :],
                                    lhsT=qk_res[:, kf,
                                                tok0 + c * 128:
                                                tok0 + (c + 1) * 128],
                                    rhs=qs,
                                    start=True, stop=True,
                                )
                            p_sb = p2p.tile([128, 2, 512], BF16, tag="p")
                            nc.scalar.activation(
                                p_sb, s_ps,
                                mybir.ActivationFunctionType.Exp,
                                scale=SCALE, bias=zeros_sb,
                            )
                            if diag0 >= 0:
                                # zero the causally-masked region of both
                                # halves with one 1024-wide bf16 multiply
                                nc.vector.tensor_mul(
                                    p_sb, p_sb, maskb_sb[:, diag0:diag0 + 2, :])
                            return p_sb

                        def pvs(h, j, t, p_sb):
                            s_ = st[h]
                            nk = 4 * j + 4
                            for e in range(2):
                                c = 2 * t + e
                                nc.tensor.matmul(
                                    s_["yps"],
                                    lhsT=v_pre[(b, h)][:, c, :],
                                    rhs=p_sb[:, e, :],
                                    start=(c == 0), stop=(c == nk - 1),
                                )
                                if c in s_["pe_rows"]:
                                    nc.tensor.matmul(
                                        s_["rps"],
                                        lhsT=onesb_sb,
                                        rhs=p_sb[:, e, :],
                                        start=(c == s_["pe_rows"][0]),
                                        stop=False,
                                    )
                                elif c % 2 == 0:
                                    with nc.allow_low_precision(
                                            reason="softmax denom bf16"):
                                        if s_["nv"] == 0:
                                            nc.vector.tensor_copy(
                                                s_["rav"], p_sb[:, e, :])
                                        else:
                                            nc.vector.tensor_add(
                                                s_["rav"], s_["rav"],
                                                p_sb[:, e, :])
                                    s_["nv"] += 1
                                else:
                                    with nc.allow_low_precision(
                                            reason="softmax denom bf16"):
                                        if s_["ng"] == 0:
                                            nc.gpsimd.tensor_copy(
                                                s_["rag"], p_sb[:, e, :])
                                        else:
                                            nc.gpsimd.tensor_add(
                                                s_["rag"], s_["rag"],
                                                p_sb[:, e, :])
                                    s_["ng"] += 1

                        def finalize(h, j):
                            s_ = st[h]
                            nc.tensor.matmul(
                                s_["rps"], lhsT=onesb_sb, rhs=s_["rav"],
                                start=(len(s_["pe_rows"]) == 0), stop=False,
                            )
                            nc.tensor.matmul(
                                s_["rps"], lhsT=onesb_sb, rhs=s_["rag"],
                                start=False, stop=True,
                            )
                            r_sb = p2rs.tile([1, 512], BF16, tag="rsb")
                            nc.vector.tensor_copy(r_sb, s_["rps"])
                            y_sb = p2y.tile([128, 512], BF16, tag="ysb")
                            nc.scalar.activation(
                                y_sb, s_["yps"],
                                mybir.ActivationFunctionType.Copy,
                                bias=0.0,
                            )
                            for e in range(2):
                                s = 2 * j + e
                                base = s * 2 * SROW + h * SROW
                                nc.sync.dma_start(
                                    out=y_loc[b][base + 128:base + 129, :],
                                    in_=r_sb[0:1, e * 256:(e + 1) * 256],
                                )
                                nc.sync.dma_start(
                                    out=y_loc[b][base:base + 128, :],
                                    in_=y_sb[:, e * 256:(e + 1) * 256],
                                )

                        def start_j(h, j):
                            s_ = st[h]
                            nk = 4 * j + 4
                            s_["j"] = j
                            s_["yps"] = p2psy.tile([128, 512], F32, tag="yps",
                                                   name=f"yps{h}")
                            s_["rps"] = p2psr.tile([1, 512], F32, tag="rps",
                                                   name=f"rps{h}")
                            s_["rav"] = p2r.tile([128, 512], BF16, tag="rav",
                                                 name=f"rav{h}")
                            s_["rag"] = p2r.tile([128, 512], BF16, tag="rag",
                                                 name=f"rag{h}")
                            s_["pe_rows"] = []
                            s_["nv"] = s_["ng"] = 0

                        for k in range(2 * len(steps)):
                            h = k % 2
                            s_ = st[h]
                            if b == 0 and k in (6, 16, 26, 36) and nwp[0] < 4:
                                load_wp_chunk(nwp[0])
                                nwp[0] += 1
                            j, t = steps[s_["idx"]]
                            s_["idx"] += 1
                            if t == 0:
                                if s_["pend"] is not None:
                                    pj, pt, pp = s_["pend"]
                                    pvs(h, pj, pt, pp)
                                    finalize(h, pj)
                                start_j(h, j)
                                s_["pend"] = (j, t, scores(h, j, t))
                            else:
                                p_new = scores(h, j, t)
                                pj, pt, pp = s_["pend"]
                                pvs(h, pj, pt, pp)
                                s_["pend"] = (j, t, p_new)
                        for h in range(HPC):
                            s_ = st[h]
                            pj, pt, pp = s_["pend"]
                            pvs(h, pj, pt, pp)
                            finalize(h, pj)
                        a2a_op(y_loc[b], y_t[b])

                    for b in range(B):
                        attn_batch(b)
                # attention pools closed: PSUM freed for the projection
                with tc.tile_pool(name="p4ps", bufs=4, space="PSUM") as p4ps:

                    def proj_pair(b, chp):
                        # 4 PSUM groups in flight: LDWEIGHTS of one group
                        # pipelines under the streams of the other three
                        grp = [(2 * chp + i, tb) for i in range(2)
                               for tb in range(2)]
                        pps = {g: p4ps.tile([128, 512], F32, tag="ops",
                                            name=f"ops{g[0] % 2}{g[1]}")
                               for g in grp}
                        for c in range(16):
                            yi = (c % 2) * 8 + c // 2   # chunk g=2s+h -> h*8+s
                            for (ch, tb) in grp:
                                nc.tensor.matmul(
                                    pps[(ch, tb)],
                                    lhsT=yts[b][:, yi, tb * 128:(tb + 1) * 128],
                                    rhs=wp_tiles[ch][:, c, :],
                                    start=(c == 0), stop=(c == 15),
                                )
                        for (ch, tb) in grp:
                            st_ = p4s.tile([128, 512], F32, tag="ost")
                            nc.vector.tensor_copy(st_, pps[(ch, tb)])
                            nc.sync.dma_start(
                                out=out[b * 256 + tb * 128:
                                        b * 256 + (tb + 1) * 128,
                                        ch * 512:(ch + 1) * 512],
                                in_=st_,
                            )

                    # b0's data arrived long ago: recv + proj b0 execute
                    # immediately, covering b1's a2a + recv latency
                    recv_batch(0)
                    for chp in range(2):
                        proj_pair(0, chp)
                    recv_batch(1)
                    for chp in range(2):
                        proj_pair(1, chp)
            p2v_ctx.__exit__(None, None, None)

    nc.compile()
    return nc


def make_in_maps(x, w_attn, w_proj):
    x = np.asarray(x, dtype=np.float32)
    w_attn = np.asarray(w_attn, dtype=np.float32)
    w_proj = np.asarray(w_proj, dtype=np.float32)
    xt = np.ascontiguousarray(x.reshape(NTOK, C).T)          # [C, NTOK]
    wp = np.ascontiguousarray(w_proj.astype(NPBF16))
    in_maps = []
    for i in range(NCORES):
        qcols = w_attn[:, FLOC * i: FLOC * (i + 1)]
        kcols = w_attn[:, C + FLOC * i: C + FLOC * (i + 1)]
        vcols = w_attn[:, 2 * C + FLOC * i: 2 * C + FLOC * (i + 1)]
        in_maps.append({
            "xt": xt,
            "wqk": np.ascontiguousarray(np.concatenate([qcols, kcols], axis=1)),
            "wv": np.ascontiguousarray(vcols),
            "wp": wp,
        })
    return in_maps


def kernel(x, w_attn, w_proj):
    global last_exec_time_ns
    if "nc" not in _cache:
        _cache["nc"] = build_nc()
    nc = _cache["nc"]
    in_maps = make_in_maps(x, w_attn, w_proj)
    res = run_bass_kernel_spmd(nc, in_maps, list(range(NCORES)))
    last_exec_time_ns = res.exec_time_ns
    return assemble([res.results[g]["out"] for g in range(NCORES)])


def assemble(outs):
    # core g's out rows: [0:256] = batch0 tokens [256g:256(g+1)],
    #                    [256:512] = batch1 tokens [256g:256(g+1)]
    full = np.empty((B, T, C), np.float32)
    for g in range(NCORES):
        for b in range(B):
            full[b, 256 * g: 256 * (g + 1), :] = outs[g][b * 256:(b + 1) * 256]
    return full

